# revision 12
# baseline (speedup 1.0000x reference)
"""Grouped linear (MoE routing) kernel for 8 Trainium2 NeuronCores.

out[n] = x[n] @ weight[g[n]].T + bias[g[n]]

Strategy: expert-parallel. group_indices is (assumed) sorted; host code
computes per-group row ranges, pads each group's rows to a common
capacity C (multiple of 128), and core g computes the dense GEMM
  out_g = x_g @ weight[g].T + bias[g]
entirely on-core with no collectives. Host gathers/scatters rows.

Per-core Bass kernel (SPMD, identical program on 8 cores):
  - x and W are bf16 (PSUM accumulation stays fp32). The rel-err budget
    (2e-2) dwarfs bf16 GEMM error (~2.4e-3 measured for this size), the
    PE runs bf16 at the same 1 col/cycle as fp32r, and halving the bytes
    halves the W cold-load (8 MB, ~22 us at the ~358 GB/s per-NC HBM
    limit) and the streamed x tiles (0.5 MB each).
  - W^T [D_IN, D_OUT] stays resident in SBUF, DMA'd in 512-col n-slices
    (the first one in two ko-halves) so the PE's first matmul group can
    start after ~1.25 MB has landed instead of ~4 MB.
  - Phase A runs n-outer over 4 resident x tiles, paced so each W
    n-slice lands just before the PE needs it; phase B streams the
    remaining m-tiles m-outer with double-buffered x.
  - PSUM [128, 512] accumulates over the 16 k-subtiles; bias add happens
    on the VectorE during PSUM->SBUF eviction.
"""

import math
import sys

for _p in ("/opt/trn_rl_repo", "/root/.axon_site/_ro/trn_rl_repo"):
    if _p not in sys.path:
        sys.path.append(_p)

import ml_dtypes
import numpy as np

from concourse import bacc, mybir, tile
from concourse.bass_utils import run_bass_kernel_spmd

P = 128
D_IN = 2048
D_OUT = 2048
KO = D_IN // P  # 16 k-subtiles
N_TILE = 512
N_TILES = D_OUT // N_TILE  # 4
N_BLK = D_OUT // P  # 16 feature blocks for the overflow segment
NUM_GROUPS = 8
N_CORES = 8

# Overflow-segment length (rows handled moving-x); set by shard_inputs.
OV_R = 164

_nc_cache: dict = {}


def build_program(C: int, repeat: int = 1, inner: str = "n"):
    """Build + compile the per-core Bass program for row capacity C."""
    key = (C, repeat, inner)
    if key in _nc_cache:
        return _nc_cache[key]
    assert C % P == 0
    m_tiles = C // P
    f32 = mybir.dt.float32
    bf16 = mybir.dt.bfloat16

    nc = bacc.Bacc(
        "TRN2", target_bir_lowering=False, debug=False, num_devices=N_CORES
    )
    # Blocked HBM layouts (prepared host-side) so every DMA moves large
    # contiguous per-partition runs:
    #   xT[m, kp, ko, j]  = x[m*128+j, ko*128+kp]   (4 KB/partition/DMA)
    #   wT[n, kp, ko, nn] = W^T[ko*128+kp, n*512+nn] (16 KB/partition/DMA)
    xT = nc.dram_tensor(
        "xT", [m_tiles, P, KO, P], bf16, kind="ExternalInput"
    ).ap()
    wT = nc.dram_tensor(
        "wT", [N_TILES, P, KO, N_TILE], bf16, kind="ExternalInput"
    ).ap()
    bb = nc.dram_tensor("bb", [P, D_OUT], f32, kind="ExternalInput").ap()
    out = nc.dram_tensor("out", [C, D_OUT], f32, kind="ExternalOutput").ap()

    # Phase A: the first PH_A m-tiles are processed n-outer while W^T
    # streams in n-major slices — the PE's in-order stream always has
    # work whose W slice has already arrived. Phase B (steady state):
    # W is resident, m-tiles stream m-outer.
    ph_a = min(4, m_tiles)

    with tile.TileContext(nc) as tc:
        with (
            tc.tile_pool(name="wpool", bufs=1) as wpool,
            tc.tile_pool(name="cpool", bufs=1) as cpool,
            tc.tile_pool(name="wmpool", bufs=1) as wmpool,
            tc.tile_pool(name="xapool", bufs=1) as xapool,
            tc.tile_pool(name="xpool", bufs=2) as xpool,
            tc.tile_pool(name="opool", bufs=3) as opool,
            tc.tile_pool(name="ohpool", bufs=4) as ohpool,
            tc.tile_pool(name="ofpool", bufs=2) as ofpool,
            tc.tile_pool(name="pspool", bufs=8, space="PSUM") as pspool,
        ):
            w_sb = wpool.tile([P, N_TILES, KO, N_TILE], bf16)
            b_sb = cpool.tile([P, D_OUT], f32)
            xa_sb = xapool.tile([P, ph_a, KO, P], bf16)
            warm_sb = wmpool.tile([P, 192], bf16)

            # Startup DMAs. Two pacing mechanisms (cost model + HW): each
            # dma_start occupies the HWDGE issue ring ~625 ns (FIFO), and
            # the moves share one ~358 GB/s HBM pipe, so pieces are sized
            # ~0.25-0.5 MB (issue time ~ move time) and kept few: the
            # first matmul group can start at ~3.5 us, and each W n-slice
            # lands just before the n-outer phase-A pass that needs it.
            nc.sync.dma_start(xa_sb[:, 0, 0:8], xT[0, :, 0:8])
            for q in range(4):
                ks = slice(4 * q, 4 * (q + 1))
                nc.sync.dma_start(w_sb[:, 0, ks], wT[0, :, ks])
                if q == 0:
                    nc.sync.dma_start(xa_sb[:, 0, 8:16], xT[0, :, 8:16])
                elif q < 3:
                    nc.sync.dma_start(xa_sb[:, q], xT[q])
            if ph_a > 3:
                nc.sync.dma_start(xa_sb[:, 3], xT[3])
            nc.sync.dma_start(w_sb[:, 1], wT[1])
            nc.sync.dma_start(b_sb[:], bb[:])
            for n in range(2, N_TILES):
                nc.sync.dma_start(w_sb[:, n], wT[n])

            # PE prewarm: the HAM throttle holds the PE at 1.2 GHz for
            # its first ~3.4 us of activity after idle. A few tiny
            # matmuls over a memset tile start that window ticking while
            # the first x/W DMAs are still in flight, so the real
            # matmuls run closer to 2.4 GHz from the start. The psum
            # group is never read; the pool recycles its bank.
            nc.vector.memset(warm_sb[:], 0.0)
            ps_w = pspool.tile([P, N_TILE], f32, tag="ps")
            for i in range(8):
                nc.tensor.matmul(
                    ps_w[:, 0:64],
                    warm_sb[:, 0:128],
                    warm_sb[:, 128:192],
                    start=(i == 0),
                    stop=(i == 7),
                )

            def evict(ps, m, n):
                ms = slice(m * P, (m + 1) * P)
                ns = slice(n * N_TILE, (n + 1) * N_TILE)
                o_sb = opool.tile([P, N_TILE], f32, tag="o")
                nc.vector.tensor_add(o_sb[:], ps, b_sb[:, ns])
                nc.sync.dma_start(out[ms, ns], o_sb[:])

            def evict_split(ps, m, n, pieces=4):
                # halved pieces so the very last out DMA starts sooner —
                # trims the post-matmul drain on the final tile
                ms = slice(m * P, (m + 1) * P)
                w = N_TILE // pieces
                for j in range(pieces):
                    ns = slice(n * N_TILE + j * w, n * N_TILE + (j + 1) * w)
                    o_sb = ohpool.tile([P, w], f32, tag="oh")
                    nc.vector.tensor_add(o_sb[:], ps[:, j * w : (j + 1) * w], b_sb[:, ns])
                    nc.sync.dma_start(out[ms, ns], o_sb[:])

            def do_group(x_tile, m, n, o_full=None, split=False):
                ps = pspool.tile([P, N_TILE], f32, tag="ps")
                for ko in range(KO):
                    nc.tensor.matmul(
                        ps,
                        x_tile[:, ko],
                        w_sb[:, n, ko],
                        start=(ko == 0),
                        stop=(ko == KO - 1),
                    )
                if split:
                    evict_split(ps, m, n)
                elif o_full is None:
                    evict(ps, m, n)
                else:
                    ns = slice(n * N_TILE, (n + 1) * N_TILE)
                    nc.vector.tensor_add(o_full[:, ns], ps, b_sb[:, ns])

            for rep in range(repeat):
                if rep == 0:
                    # phase A: n-outer over the resident x tiles
                    for n in range(N_TILES):
                        for m in range(ph_a):
                            do_group(xa_sb[:, m], m, n)
                    b_start = ph_a
                else:
                    b_start = 0
                # phase B: steady-state streaming; full-row out tiles so the
                # out DMA writes 8 KB/partition contiguous
                for m in range(b_start, m_tiles):
                    x_sb = xpool.tile([P, KO, P], bf16, tag="x")
                    nc.sync.dma_start(x_sb[:], xT[m])
                    if m == m_tiles - 1:
                        # last tile: per-slice eviction so the final out DMA
                        # doesn't serialize behind all 4 bias-adds
                        for n in range(N_TILES):
                            do_group(x_sb, m, n)
                    else:
                        o_full = ofpool.tile([P, D_OUT], f32, tag="of")
                        for n in range(N_TILES):
                            do_group(x_sb, m, n, o_full=o_full)
                        nc.sync.dma_start(
                            out[m * P : (m + 1) * P, :], o_full[:]
                        )

    nc.compile()
    _nc_cache[key] = nc
    return nc


def shard_inputs(x, weight, bias, group_indices):
    """Host-side expert-parallel sharding. Returns (in_maps, perm, offsets,
    counts, C)."""
    n_rows = x.shape[0]
    gi = np.asarray(group_indices)
    # Sorted in the reference's setup; stable argsort keeps it general and
    # is nearly free when already sorted.
    perm = np.argsort(gi, kind="stable")
    counts = np.bincount(gi, minlength=NUM_GROUPS).astype(np.int64)
    offsets = np.zeros(NUM_GROUPS + 1, dtype=np.int64)
    np.cumsum(counts, out=offsets[1:])
    C = max(P, int(math.ceil(counts.max() / P)) * P)

    x_sorted = x[perm] if not np.array_equal(perm, np.arange(n_rows)) else x
    m_tiles = C // P
    in_maps = []
    for g in range(NUM_GROUPS):
        ng = int(counts[g])
        xg = np.zeros((C, D_IN), dtype=np.float32)
        xg[:ng] = x_sorted[offsets[g] : offsets[g] + ng]
        # blocked layouts — see build_program
        xb = np.ascontiguousarray(
            xg.astype(ml_dtypes.bfloat16)
            .reshape(m_tiles, P, KO, P)
            .transpose(0, 3, 2, 1)
        )
        wb = np.ascontiguousarray(
            weight[g]
            .astype(ml_dtypes.bfloat16)
            .T.reshape(KO, P, N_TILES, N_TILE)
            .transpose(2, 1, 0, 3)
        )
        in_maps.append(
            {
                "xT": xb,
                "wT": wb,
                "bb": np.ascontiguousarray(
                    np.broadcast_to(bias[g], (P, D_OUT))
                ),
            }
        )
    return in_maps, perm, offsets, counts, C


def unshard_output(results, perm, offsets, counts, n_rows):
    out = np.empty((n_rows, D_OUT), dtype=np.float32)
    for g in range(NUM_GROUPS):
        ng = int(counts[g])
        out[perm[offsets[g] : offsets[g] + ng]] = results[g]["out"][:ng]
    return out


def kernel(x, weight, bias, group_indices):
    x = np.asarray(x, dtype=np.float32)
    weight = np.asarray(weight, dtype=np.float32)
    bias = np.asarray(bias, dtype=np.float32)
    group_indices = np.asarray(group_indices)
    assert x.shape[1] == D_IN and weight.shape == (NUM_GROUPS, D_OUT, D_IN)

    in_maps, perm, offsets, counts, C = shard_inputs(
        x, weight, bias, group_indices
    )
    nc = build_program(C)
    res = run_bass_kernel_spmd(nc, in_maps, core_ids=list(range(N_CORES)))
    return unshard_output(res.results, perm, offsets, counts, x.shape[0])
